# revision 6
# baseline (speedup 1.0000x reference)
"""PixelMixer Trainium2 kernel.

x: [8, 512, 512, 60] f32. Channel c (residue r = c % 5):
  r=0: out[h,w] = x[h, w+1]   (circular)
  r=1: out[h,w] = x[h, w-1]
  r=2: out[h,w] = x[h+1, w]
  r=3: out[h,w] = x[h-1, w]
  r=4: out[h,w] = x[h, w]

Sharding: batch-parallel, image b -> core b (no cross-core traffic).

Per-core layout: partition p in [0,128) holds rows [4p, 4p+4).
W is processed in 16 chunks of 32 output pixels. Each chunk loads 34
pixels (1-pixel halo each side) per row into SBUF. The two H-halo rows
(row 4p+4 and row 4p-1) are staged either by partition-shifted
SBUF->SBUF DMAs (variant "dma") or by TensorE one-hot shift matmuls
into PSUM (variant "pe", keeps staging traffic off the DMA fabric).
7 strided DVE copies assemble the interleaved output channels.
Free-dim trick: within a row, {pixel, channel-group} indices merge into
one stride-5 axis since 60 = 12*5.
"""

import numpy as np

import concourse.bass as bass
import concourse.bacc as bacc
import concourse.tile as tile
from concourse import mybir
from concourse import bass_utils

H, W, C = 512, 512, 60
NP = 128           # partitions
R = H // NP        # 4 rows per partition
PIX = 32           # output pixels per chunk
NCH = W // PIX     # 16 chunks
UIN = 34 * (C // 5)    # 408
UOUT = PIX * (C // 5)  # 384
FIN = 34 * C       # 2040 f32 per row-slot (34 pixels)
FOUT = PIX * C     # 1920

VARIANT = "pe"     # "dma" or "pe"

_NC_CACHE = {}


def shift_mats():
    # out = lhsT.T @ rhs ; sn: out[p]=in[p+1 mod 128], sp: out[p]=in[p-1]
    eye = np.eye(NP, dtype=np.float32)
    sn = np.roll(eye, 1, axis=0)
    sp = np.roll(eye, -1, axis=0)
    return sn, sp


def _build_nc(variant=VARIANT):
    if variant in _NC_CACHE:
        return _NC_CACHE[variant]
    nc = bacc.Bacc("TRN2", target_bir_lowering=False, debug=False,
                   enable_asserts=False)
    f32 = mybir.dt.float32
    x = nc.dram_tensor("x", [H, W, C], f32, kind="ExternalInput").ap()
    y = nc.dram_tensor("y", [H, W, C], f32, kind="ExternalOutput").ap()
    if variant == "pe":
        sn_d = nc.dram_tensor("sn", [NP, NP], f32, kind="ExternalInput").ap()
        sp_d = nc.dram_tensor("sp", [NP, NP], f32, kind="ExternalInput").ap()
    xr = x.rearrange("(p r) w c -> p r (w c)", p=NP)
    yr = y.rearrange("(p r) w c -> p r (w c)", p=NP)

    with tile.TileContext(nc) as tc:
        with tc.tile_pool(name="mpool", bufs=2) as mpool, \
             tc.tile_pool(name="hpool", bufs=2) as hpool, \
             tc.tile_pool(name="opool", bufs=2) as opool, \
             tc.tile_pool(name="cpool", bufs=1) as cpool, \
             tc.tile_pool(name="ppool", bufs=1, space="PSUM") as ppool:
            if variant == "pe":
                snt = cpool.tile([NP, NP], f32, name="snt")
                spt = cpool.tile([NP, NP], f32, name="spt")
                nc.sync.dma_start(snt[:], sn_d[:])
                nc.sync.dma_start(spt[:], sp_d[:])

            for k in range(NCH):
                # in-tile: [part, row-slot 0..3, u=pixslot*12+grp, res]
                mt = mpool.tile([NP, R, UIN, 5], f32, name=f"mt{k}", tag="mt")
                ot = opool.tile([NP, R, UOUT, 5], f32, name=f"ot{k}", tag="ot")
                mtf = mt.rearrange("p r u c -> p r (u c)")
                otf = ot.rearrange("p r u c -> p r (u c)")

                # ---- load 34-pixel band (pixels 32k-1 .. 32k+32, circular)
                a = (PIX * k - 1) * C
                if k == 0:
                    nc.sync.dma_start(mtf[:, :, C:FIN], xr[:, :, 0:FIN - C])
                    nc.sync.dma_start(mtf[:, :, 0:C],
                                      xr[:, :, (W - 1) * C:W * C])
                elif k == NCH - 1:
                    nc.sync.dma_start(mtf[:, :, 0:FIN - C],
                                      xr[:, :, a:a + FIN - C])
                    nc.sync.dma_start(mtf[:, :, FIN - C:FIN], xr[:, :, 0:C])
                else:
                    nc.sync.dma_start(mtf[:, :, :], xr[:, :, a:a + FIN])

                # ---- stage H-halo rows
                if variant == "dma":
                    ht = hpool.tile([NP, 2, UIN, 5], f32, name=f"ht{k}",
                                    tag="ht")
                    htf = ht.rearrange("p s u c -> p s (u c)")
                    # slot 0: next row (4p+4) = partition p+1's row-slot 0
                    nc.sync.dma_start(htf[0:NP - 1, 0, :], mtf[1:NP, 0, :])
                    nc.sync.dma_start(htf[NP - 1:NP, 0, :], mtf[0:1, 0, :])
                    # slot 1: prev row (4p-1) = partition p-1's row-slot 3
                    nc.sync.dma_start(htf[1:NP, 1, :],
                                      mtf[0:NP - 1, R - 1, :])
                    nc.sync.dma_start(htf[0:1, 1, :],
                                      mtf[NP - 1:NP, R - 1, :])
                    nx = ht[:, 0, :, :]   # [NP, UIN, 5]
                    pv = ht[:, 1, :, :]
                else:
                    pn = ppool.tile([NP, 2048], f32, name=f"pn{k}", tag="pn")
                    pp = ppool.tile([NP, 2048], f32, name=f"pp{k}", tag="pp")
                    for j in range(4):
                        sz = min(512, FIN - 512 * j)
                        nc.tensor.matmul(pn[:, 512 * j:512 * j + sz], snt[:],
                                         mtf[:, 0, 512 * j:512 * j + sz])
                        nc.tensor.matmul(pp[:, 512 * j:512 * j + sz], spt[:],
                                         mtf[:, R - 1, 512 * j:512 * j + sz])
                    nx = pn[:, 0:FIN].rearrange("p (u c) -> p u c", c=5)
                    pv = pp[:, 0:FIN].rearrange("p (u c) -> p u c", c=5)

                # ---- assemble output residues (DVE strided copies)
                # r=0: w+1 -> in pixel-slot j+2 -> u offset +24
                nc.vector.tensor_copy(ot[:, :, :, 0], mt[:, :, 24:24 + UOUT, 0])
                # r=1: w-1 -> pixel-slot j -> u offset 0
                nc.vector.tensor_copy(ot[:, :, :, 1], mt[:, :, 0:UOUT, 1])
                # r=4: same pixel -> slot j+1 -> u offset +12
                nc.vector.tensor_copy(ot[:, :, :, 4], mt[:, :, 12:12 + UOUT, 4])
                # r=2: h+1 -> rows 0..2 from in rows 1..3
                nc.vector.tensor_copy(ot[:, 0:R - 1, :, 2],
                                      mt[:, 1:R, 12:12 + UOUT, 2])
                # r=2 row 3 from next-row halo
                nc.vector.tensor_copy(ot[:, R - 1, :, 2], nx[:, 12:12 + UOUT, 2])
                # r=3: h-1 -> rows 1..3 from in rows 0..2
                nc.vector.tensor_copy(ot[:, 1:R, :, 3],
                                      mt[:, 0:R - 1, 12:12 + UOUT, 3])
                # r=3 row 0 from prev-row halo
                nc.vector.tensor_copy(ot[:, 0, :, 3], pv[:, 12:12 + UOUT, 3])

                # ---- store
                nc.sync.dma_start(yr[:, :, k * FOUT:(k + 1) * FOUT],
                                  otf[:, :, :])

    nc.finalize()
    _NC_CACHE[variant] = nc
    return nc


def make_in_maps(x, variant=VARIANT):
    B = x.shape[0]
    maps = [{"x": x[b]} for b in range(B)]
    if variant == "pe":
        sn, sp = shift_mats()
        for m in maps:
            m["sn"] = sn
            m["sp"] = sp
    return maps


def run(x: np.ndarray, variant=VARIANT):
    """Returns (out [B,H,W,C], BassKernelResults)."""
    x = np.ascontiguousarray(x, dtype=np.float32)
    B = x.shape[0]
    nc = _build_nc(variant)
    res = bass_utils.run_bass_kernel_spmd(nc, make_in_maps(x, variant),
                                          core_ids=list(range(B)))
    out = np.stack([r["y"] for r in res.results], axis=0)
    return out, res


def kernel(x: np.ndarray) -> np.ndarray:
    out, _ = run(x)
    return out


# revision 13
# speedup vs baseline: 17.9213x; 17.9213x over previous
"""PixelMixer Trainium2 kernel.

x: [8, 512, 512, 60] f32. Channel c (residue r = c % 5):
  r=0: out[h,w] = x[h, w+1]   (circular)
  r=1: out[h,w] = x[h, w-1]
  r=2: out[h,w] = x[h+1, w]
  r=3: out[h,w] = x[h-1, w]
  r=4: out[h,w] = x[h, w]

Sharding: batch-parallel, image b -> core b (no cross-core traffic).

Per-core layout: partition p in [0,128) holds rows [4p, 4p+4).
W is processed in 16 chunks of 32 output pixels. Each chunk loads 34
pixels (1-pixel halo each side) per row into SBUF. The two H-halo rows
(row 4p+4 and row 4p-1) are staged either by partition-shifted
SBUF->SBUF DMAs (variant "dma") or by TensorE one-hot shift matmuls
into PSUM (variant "pe", keeps staging traffic off the DMA fabric).
7 strided DVE copies assemble the interleaved output channels.
Free-dim trick: within a row, {pixel, channel-group} indices merge into
one stride-5 axis since 60 = 12*5.
"""

import numpy as np

import concourse.bass as bass
import concourse.bacc as bacc
import concourse.tile as tile
from concourse import mybir
from concourse import bass_utils

H, W, C = 512, 512, 60
NP = 128           # partitions
R = H // NP        # 4 rows per partition
PIX = 32           # output pixels per chunk
NCH = W // PIX     # 16 chunks
UIN = 34 * (C // 5)    # 408
UOUT = PIX * (C // 5)  # 384
FIN = 34 * C       # 2040 f32 per row-slot (34 pixels)
FOUT = PIX * C     # 1920

VARIANT = "pe"     # "dma" or "pe"

_NC_CACHE = {}


def shift_mats():
    # out = lhsT.T @ rhs ; sn: out[p]=in[p+1 mod 128], sp: out[p]=in[p-1]
    eye = np.eye(NP, dtype=np.float32)
    sn = np.roll(eye, 1, axis=0)
    sp = np.roll(eye, -1, axis=0)
    return sn, sp


def _build_v3(nc, reps, store_on_act=True):
    """No W-halo loads: boundary pixels come from neighbor chunk tiles
    (deferred r=0 tail copy + one-iteration-deferred store). Stores issue
    on the ACT HWDGE ring when store_on_act, keeping the SP ring for loads.
    """
    f32 = mybir.dt.float32
    G = C // 5  # 12
    x = nc.dram_tensor("x", [H, W, C], f32, kind="ExternalInput").ap()
    y = nc.dram_tensor("y", [H, W, C], f32, kind="ExternalOutput").ap()
    sn_d = nc.dram_tensor("sn", [NP, NP], f32, kind="ExternalInput").ap()
    sp_d = nc.dram_tensor("sp", [NP, NP], f32, kind="ExternalInput").ap()
    xr = x.rearrange("(p r) w c -> p r (w c)", p=NP)
    yr = y.rearrange("(p r) w c -> p r (w c)", p=NP)
    st = nc.scalar if store_on_act else nc.sync

    with tile.TileContext(nc) as tc:
        with tc.tile_pool(name="mpool", bufs=3) as mpool, \
             tc.tile_pool(name="opool", bufs=2) as opool, \
             tc.tile_pool(name="cpool", bufs=1) as cpool, \
             tc.tile_pool(name="ppool", bufs=1, space="PSUM") as ppool:
            snt = cpool.tile([NP, NP], f32, name="snt")
            spt = cpool.tile([NP, NP], f32, name="spt")
            wl = cpool.tile([NP, R, G, 5], f32, name="wl")  # w=0 col
            wr = cpool.tile([NP, R, G, 5], f32, name="wr")  # w=511 col
            nc.sync.dma_start(snt[:], sn_d[:])
            nc.sync.dma_start(spt[:], sp_d[:])
            nc.sync.dma_start(wl.rearrange("p r g c -> p r (g c)"),
                              xr[:, :, 0:C])
            nc.sync.dma_start(wr.rearrange("p r g c -> p r (g c)"),
                              xr[:, :, (W - 1) * C:W * C])

            for rep in range(reps):
                prev_mt = prev_ot = prev_otf = None
                for k in range(NCH):
                    mt = mpool.tile([NP, R, UOUT, 5], f32,
                                    name=f"m3_{rep}_{k}", tag="mt")
                    ot = opool.tile([NP, R, UOUT, 5], f32,
                                    name=f"o3_{rep}_{k}", tag="ot")
                    mtf = mt.rearrange("p r u c -> p r (u c)")
                    otf = ot.rearrange("p r u c -> p r (u c)")
                    nc.sync.dma_start(mtf[:, :, :],
                                      xr[:, :, k * FOUT:(k + 1) * FOUT])

                    pn = ppool.tile([NP, 2048], f32, name=f"pn3_{rep}_{k}",
                                    tag="pn")
                    pp = ppool.tile([NP, 2048], f32, name=f"pp3_{rep}_{k}",
                                    tag="pp")
                    for j in range(0, FOUT, 512):
                        sz = min(512, FOUT - j)
                        nc.tensor.matmul(pn[:, j:j + sz], snt[:],
                                         mtf[:, 0, j:j + sz])
                        nc.tensor.matmul(pp[:, j:j + sz], spt[:],
                                         mtf[:, R - 1, j:j + sz])
                    nx = pn[:, 0:FOUT].rearrange("p (u c) -> p u c", c=5)
                    pv = pp[:, 0:FOUT].rearrange("p (u c) -> p u c", c=5)

                    U = UOUT
                    # r=0 (w+1): pixels 0..30 from own tile; tail deferred
                    nc.vector.tensor_copy(ot[:, :, 0:U - G, 0],
                                          mt[:, :, G:U, 0])
                    # r=1 (w-1): pixels 1..31 from own; pixel 0 from prev/wr
                    nc.vector.tensor_copy(ot[:, :, G:U, 1],
                                          mt[:, :, 0:U - G, 1])
                    if k == 0:
                        nc.vector.tensor_copy(ot[:, :, 0:G, 1],
                                              wr[:, :, :, 1])
                    else:
                        nc.vector.tensor_copy(ot[:, :, 0:G, 1],
                                              prev_mt[:, :, U - G:U, 1])
                    nc.vector.tensor_copy(ot[:, :, :, 4], mt[:, :, :, 4])
                    nc.vector.tensor_copy(ot[:, 0:R - 1, :, 2],
                                          mt[:, 1:R, :, 2])
                    nc.vector.tensor_copy(ot[:, R - 1, :, 2], nx[:, :, 2])
                    nc.vector.tensor_copy(ot[:, 1:R, :, 3],
                                          mt[:, 0:R - 1, :, 3])
                    nc.vector.tensor_copy(ot[:, 0, :, 3], pv[:, :, 3])

                    if prev_ot is not None:
                        nc.vector.tensor_copy(prev_ot[:, :, U - G:U, 0],
                                              mt[:, :, 0:G, 0])
                        st.dma_start(yr[:, :, (k - 1) * FOUT:k * FOUT],
                                     prev_otf[:, :, :])
                    prev_mt, prev_ot, prev_otf = mt, ot, otf

                nc.vector.tensor_copy(prev_ot[:, :, UOUT - G:UOUT, 0],
                                      wl[:, :, :, 0])
                st.dma_start(yr[:, :, (NCH - 1) * FOUT:NCH * FOUT],
                             prev_otf[:, :, :])


def _build_nc(variant=VARIANT, reps=1):
    key = (variant, reps)
    if key in _NC_CACHE:
        return _NC_CACHE[key]
    nc = bacc.Bacc("TRN2", target_bir_lowering=False, debug=False,
                   enable_asserts=False)
    if variant.startswith("v3"):
        _build_v3(nc, reps, store_on_act=(variant == "v3"))
        nc.finalize()
        _NC_CACHE[key] = nc
        return nc
    f32 = mybir.dt.float32
    x = nc.dram_tensor("x", [H, W, C], f32, kind="ExternalInput").ap()
    y = nc.dram_tensor("y", [H, W, C], f32, kind="ExternalOutput").ap()
    if variant == "pe":
        sn_d = nc.dram_tensor("sn", [NP, NP], f32, kind="ExternalInput").ap()
        sp_d = nc.dram_tensor("sp", [NP, NP], f32, kind="ExternalInput").ap()
    xr = x.rearrange("(p r) w c -> p r (w c)", p=NP)
    yr = y.rearrange("(p r) w c -> p r (w c)", p=NP)

    with tile.TileContext(nc) as tc:
        with tc.tile_pool(name="mpool", bufs=2) as mpool, \
             tc.tile_pool(name="hpool", bufs=2) as hpool, \
             tc.tile_pool(name="opool", bufs=2) as opool, \
             tc.tile_pool(name="cpool", bufs=1) as cpool, \
             tc.tile_pool(name="ppool", bufs=1, space="PSUM") as ppool:
            if variant == "pe":
                snt = cpool.tile([NP, NP], f32, name="snt")
                spt = cpool.tile([NP, NP], f32, name="spt")
                nc.sync.dma_start(snt[:], sn_d[:])
                nc.sync.dma_start(spt[:], sp_d[:])

            for rep in range(reps):
              for k in range(NCH):
                # in-tile: [part, row-slot 0..3, u=pixslot*12+grp, res]
                mt = mpool.tile([NP, R, UIN, 5], f32, name=f"mt{rep}_{k}",
                                tag="mt")
                ot = opool.tile([NP, R, UOUT, 5], f32, name=f"ot{rep}_{k}",
                                tag="ot")
                mtf = mt.rearrange("p r u c -> p r (u c)")
                otf = ot.rearrange("p r u c -> p r (u c)")

                # ---- load 34-pixel band (pixels 32k-1 .. 32k+32, circular)
                a = (PIX * k - 1) * C
                if k == 0:
                    nc.sync.dma_start(mtf[:, :, C:FIN], xr[:, :, 0:FIN - C])
                    nc.sync.dma_start(mtf[:, :, 0:C],
                                      xr[:, :, (W - 1) * C:W * C])
                elif k == NCH - 1:
                    nc.sync.dma_start(mtf[:, :, 0:FIN - C],
                                      xr[:, :, a:a + FIN - C])
                    nc.sync.dma_start(mtf[:, :, FIN - C:FIN], xr[:, :, 0:C])
                else:
                    nc.sync.dma_start(mtf[:, :, :], xr[:, :, a:a + FIN])

                # ---- stage H-halo rows
                if variant == "dma":
                    ht = hpool.tile([NP, 2, UIN, 5], f32, name=f"ht{rep}_{k}",
                                    tag="ht")
                    htf = ht.rearrange("p s u c -> p s (u c)")
                    # slot 0: next row (4p+4) = partition p+1's row-slot 0
                    nc.sync.dma_start(htf[0:NP - 1, 0, :], mtf[1:NP, 0, :])
                    nc.sync.dma_start(htf[NP - 1:NP, 0, :], mtf[0:1, 0, :])
                    # slot 1: prev row (4p-1) = partition p-1's row-slot 3
                    nc.sync.dma_start(htf[1:NP, 1, :],
                                      mtf[0:NP - 1, R - 1, :])
                    nc.sync.dma_start(htf[0:1, 1, :],
                                      mtf[NP - 1:NP, R - 1, :])
                    nx = ht[:, 0, :, :]   # [NP, UIN, 5]
                    pv = ht[:, 1, :, :]
                else:
                    pn = ppool.tile([NP, 2048], f32, name=f"pn{rep}_{k}",
                                    tag="pn")
                    pp = ppool.tile([NP, 2048], f32, name=f"pp{rep}_{k}",
                                    tag="pp")
                    for j in range(4):
                        sz = min(512, FIN - 512 * j)
                        nc.tensor.matmul(pn[:, 512 * j:512 * j + sz], snt[:],
                                         mtf[:, 0, 512 * j:512 * j + sz])
                        nc.tensor.matmul(pp[:, 512 * j:512 * j + sz], spt[:],
                                         mtf[:, R - 1, 512 * j:512 * j + sz])
                    nx = pn[:, 0:FIN].rearrange("p (u c) -> p u c", c=5)
                    pv = pp[:, 0:FIN].rearrange("p (u c) -> p u c", c=5)

                # ---- assemble output residues (DVE strided copies)
                # r=0: w+1 -> in pixel-slot j+2 -> u offset +24
                nc.vector.tensor_copy(ot[:, :, :, 0], mt[:, :, 24:24 + UOUT, 0])
                # r=1: w-1 -> pixel-slot j -> u offset 0
                nc.vector.tensor_copy(ot[:, :, :, 1], mt[:, :, 0:UOUT, 1])
                # r=4: same pixel -> slot j+1 -> u offset +12
                nc.vector.tensor_copy(ot[:, :, :, 4], mt[:, :, 12:12 + UOUT, 4])
                # r=2: h+1 -> rows 0..2 from in rows 1..3
                nc.vector.tensor_copy(ot[:, 0:R - 1, :, 2],
                                      mt[:, 1:R, 12:12 + UOUT, 2])
                # r=2 row 3 from next-row halo
                nc.vector.tensor_copy(ot[:, R - 1, :, 2], nx[:, 12:12 + UOUT, 2])
                # r=3: h-1 -> rows 1..3 from in rows 0..2
                nc.vector.tensor_copy(ot[:, 1:R, :, 3],
                                      mt[:, 0:R - 1, 12:12 + UOUT, 3])
                # r=3 row 0 from prev-row halo
                nc.vector.tensor_copy(ot[:, 0, :, 3], pv[:, 12:12 + UOUT, 3])

                # ---- store
                nc.sync.dma_start(yr[:, :, k * FOUT:(k + 1) * FOUT],
                                  otf[:, :, :])

    nc.finalize()
    _NC_CACHE[key] = nc
    return nc


def make_in_maps(x, variant=VARIANT):
    B = x.shape[0]
    maps = [{"x": x[b]} for b in range(B)]
    if variant == "pe" or variant.startswith("v3"):
        sn, sp = shift_mats()
        for m in maps:
            m["sn"] = sn
            m["sp"] = sp
    return maps


def run(x: np.ndarray, variant=VARIANT):
    """Returns (out [B,H,W,C], BassKernelResults)."""
    x = np.ascontiguousarray(x, dtype=np.float32)
    B = x.shape[0]
    nc = _build_nc(variant)
    res = bass_utils.run_bass_kernel_spmd(nc, make_in_maps(x, variant),
                                          core_ids=list(range(B)))
    out = np.stack([r["y"] for r in res.results], axis=0)
    return out, res


def kernel(x: np.ndarray) -> np.ndarray:
    out, _ = run(x)
    return out


# revision 32
# speedup vs baseline: 21.2291x; 1.1846x over previous
"""PixelMixer Trainium2 kernel.

x: [8, 512, 512, 60] f32. Channel c (residue r = c % 5):
  r=0: out[h,w] = x[h, w+1]   (circular)
  r=1: out[h,w] = x[h, w-1]
  r=2: out[h,w] = x[h+1, w]
  r=3: out[h,w] = x[h-1, w]
  r=4: out[h,w] = x[h, w]

Sharding: batch-parallel, image b -> core b (no cross-core traffic).

Per-core layout: partition p in [0,128) holds rows [4p, 4p+4).
W is processed in 16 chunks of 32 pixels. H-halo rows (4p+4, 4p-1) are
produced by TensorE one-hot shift matmuls into PSUM (DVE reads PSUM
directly), keeping staging traffic off the DMA fabric. Strided DVE
copies assemble the interleaved output channels; within a row, {pixel,
channel-group} indices merge into one stride-5 axis since 60 = 12*5.

Default variant "v3sp": no W-halo re-reads -- chunk loads are exactly
32 pixels; the circular w+-1 boundary columns come from the neighbor
chunk's tile (r=0 tail copy + store deferred one iteration) and from
two persistent w=0/w=511 column tiles. All DMAs on the SP HWDGE ring.
Measured ~310 us/core on 8 cores, at the pure load+store roofline
(125.8 MB/core, ~3.25 TB/s device aggregate).
"""

import numpy as np

import concourse.bass as bass
import concourse.bacc as bacc
import concourse.tile as tile
from concourse import mybir
from concourse import bass_utils

H, W, C = 512, 512, 60
NP = 128           # partitions
R = H // NP        # 4 rows per partition
PIX = 32           # output pixels per chunk
NCH = W // PIX     # 16 chunks
UIN = 34 * (C // 5)    # 408
UOUT = PIX * (C // 5)  # 384
FIN = 34 * C       # 2040 f32 per row-slot (34 pixels)
FOUT = PIX * C     # 1920

VARIANT = "v3sp"   # "dma", "pe", "v3" (ACT-ring stores), "v3sp"

_NC_CACHE = {}


def shift_mats():
    # out = lhsT.T @ rhs ; sn: out[p]=in[p+1 mod 128], sp: out[p]=in[p-1]
    eye = np.eye(NP, dtype=np.float32)
    sn = np.roll(eye, 1, axis=0)
    sp = np.roll(eye, -1, axis=0)
    return sn, sp


def _build_v3(nc, reps, mode="sp", mbufs=3, obufs=2):
    """No W-halo loads: boundary pixels come from neighbor chunk tiles
    (deferred r=0 tail copy + one-iteration-deferred store).
    mode: "sp" all DMAs on SP ring; "act" stores on ACT ring;
    "alt" chunks alternate rings for both loads and stores.
    """
    f32 = mybir.dt.float32
    G = C // 5  # 12
    x = nc.dram_tensor("x", [H, W, C], f32, kind="ExternalInput").ap()
    y = nc.dram_tensor("y", [H, W, C], f32, kind="ExternalOutput").ap()
    sn_d = nc.dram_tensor("sn", [NP, NP], f32, kind="ExternalInput").ap()
    sp_d = nc.dram_tensor("sp", [NP, NP], f32, kind="ExternalInput").ap()
    xr = x.rearrange("(p r) w c -> p r (w c)", p=NP)
    yr = y.rearrange("(p r) w c -> p r (w c)", p=NP)
    def ld_eng(k):
        if mode == "alt":
            return nc.sync if k % 2 == 0 else nc.scalar
        return nc.sync

    def st_eng(k):
        if mode == "act":
            return nc.scalar
        if mode == "alt":
            return nc.scalar if k % 2 == 0 else nc.sync
        return nc.sync

    with tile.TileContext(nc) as tc:
        with tc.tile_pool(name="mpool", bufs=mbufs) as mpool, \
             tc.tile_pool(name="opool", bufs=obufs) as opool, \
             tc.tile_pool(name="cpool", bufs=1) as cpool, \
             tc.tile_pool(name="ppool", bufs=1, space="PSUM") as ppool:
            snt = cpool.tile([NP, NP], f32, name="snt")
            spt = cpool.tile([NP, NP], f32, name="spt")
            wl = cpool.tile([NP, R, G, 5], f32, name="wl")  # w=0 col
            wr = cpool.tile([NP, R, G, 5], f32, name="wr")  # w=511 col
            nc.sync.dma_start(snt[:], sn_d[:])
            nc.sync.dma_start(spt[:], sp_d[:])
            nc.sync.dma_start(wl.rearrange("p r g c -> p r (g c)"),
                              xr[:, :, 0:C])
            nc.sync.dma_start(wr.rearrange("p r g c -> p r (g c)"),
                              xr[:, :, (W - 1) * C:W * C])

            for rep in range(reps):
                prev_mt = prev_ot = prev_otf = None
                for k in range(NCH):
                    mt = mpool.tile([NP, R, UOUT, 5], f32,
                                    name=f"m3_{rep}_{k}", tag="mt")
                    ot = opool.tile([NP, R, UOUT, 5], f32,
                                    name=f"o3_{rep}_{k}", tag="ot")
                    mtf = mt.rearrange("p r u c -> p r (u c)")
                    otf = ot.rearrange("p r u c -> p r (u c)")
                    ld_eng(k).dma_start(mtf[:, :, :],
                                        xr[:, :, k * FOUT:(k + 1) * FOUT])

                    pn = ppool.tile([NP, 2048], f32, name=f"pn3_{rep}_{k}",
                                    tag="pn")
                    pp = ppool.tile([NP, 2048], f32, name=f"pp3_{rep}_{k}",
                                    tag="pp")
                    for j in range(0, FOUT, 512):
                        sz = min(512, FOUT - j)
                        nc.tensor.matmul(pn[:, j:j + sz], snt[:],
                                         mtf[:, 0, j:j + sz])
                        nc.tensor.matmul(pp[:, j:j + sz], spt[:],
                                         mtf[:, R - 1, j:j + sz])
                    nx = pn[:, 0:FOUT].rearrange("p (u c) -> p u c", c=5)
                    pv = pp[:, 0:FOUT].rearrange("p (u c) -> p u c", c=5)

                    U = UOUT
                    # r=0 (w+1): pixels 0..30 from own tile; tail deferred
                    nc.vector.tensor_copy(ot[:, :, 0:U - G, 0],
                                          mt[:, :, G:U, 0])
                    # r=1 (w-1): pixels 1..31 from own; pixel 0 from prev/wr
                    nc.vector.tensor_copy(ot[:, :, G:U, 1],
                                          mt[:, :, 0:U - G, 1])
                    if k == 0:
                        nc.vector.tensor_copy(ot[:, :, 0:G, 1],
                                              wr[:, :, :, 1])
                    else:
                        nc.vector.tensor_copy(ot[:, :, 0:G, 1],
                                              prev_mt[:, :, U - G:U, 1])
                    nc.vector.tensor_copy(ot[:, :, :, 4], mt[:, :, :, 4])
                    nc.vector.tensor_copy(ot[:, 0:R - 1, :, 2],
                                          mt[:, 1:R, :, 2])
                    nc.vector.tensor_copy(ot[:, R - 1, :, 2], nx[:, :, 2])
                    nc.vector.tensor_copy(ot[:, 1:R, :, 3],
                                          mt[:, 0:R - 1, :, 3])
                    nc.vector.tensor_copy(ot[:, 0, :, 3], pv[:, :, 3])

                    if prev_ot is not None:
                        nc.vector.tensor_copy(prev_ot[:, :, U - G:U, 0],
                                              mt[:, :, 0:G, 0])
                        st_eng(k - 1).dma_start(
                            yr[:, :, (k - 1) * FOUT:k * FOUT],
                            prev_otf[:, :, :])
                    prev_mt, prev_ot, prev_otf = mt, ot, otf

                nc.vector.tensor_copy(prev_ot[:, :, UOUT - G:UOUT, 0],
                                      wl[:, :, :, 0])
                st_eng(NCH - 1).dma_start(
                    yr[:, :, (NCH - 1) * FOUT:NCH * FOUT],
                    prev_otf[:, :, :])


def _build_nc(variant=VARIANT, reps=1):
    key = (variant, reps)
    if key in _NC_CACHE:
        return _NC_CACHE[key]
    nc = bacc.Bacc("TRN2", target_bir_lowering=False, debug=False,
                   enable_asserts=False)
    if variant.startswith("v3"):
        # NOTE: mbufs=4 / obufs=3 (187KB/partition SBUF) crashed the device
        # at runtime (NRT_EXEC_UNIT_UNRECOVERABLE); keep total <= 156KB.
        cfg = {"v3": dict(mode="act"),
               "v3sp": dict(mode="sp"),
               "v3alt": dict(mode="alt")}[variant]
        _build_v3(nc, reps, **cfg)
        nc.finalize()
        _NC_CACHE[key] = nc
        return nc
    f32 = mybir.dt.float32
    x = nc.dram_tensor("x", [H, W, C], f32, kind="ExternalInput").ap()
    y = nc.dram_tensor("y", [H, W, C], f32, kind="ExternalOutput").ap()
    if variant == "pe":
        sn_d = nc.dram_tensor("sn", [NP, NP], f32, kind="ExternalInput").ap()
        sp_d = nc.dram_tensor("sp", [NP, NP], f32, kind="ExternalInput").ap()
    xr = x.rearrange("(p r) w c -> p r (w c)", p=NP)
    yr = y.rearrange("(p r) w c -> p r (w c)", p=NP)

    with tile.TileContext(nc) as tc:
        with tc.tile_pool(name="mpool", bufs=2) as mpool, \
             tc.tile_pool(name="hpool", bufs=2) as hpool, \
             tc.tile_pool(name="opool", bufs=2) as opool, \
             tc.tile_pool(name="cpool", bufs=1) as cpool, \
             tc.tile_pool(name="ppool", bufs=1, space="PSUM") as ppool:
            if variant == "pe":
                snt = cpool.tile([NP, NP], f32, name="snt")
                spt = cpool.tile([NP, NP], f32, name="spt")
                nc.sync.dma_start(snt[:], sn_d[:])
                nc.sync.dma_start(spt[:], sp_d[:])

            for rep in range(reps):
              for k in range(NCH):
                # in-tile: [part, row-slot 0..3, u=pixslot*12+grp, res]
                mt = mpool.tile([NP, R, UIN, 5], f32, name=f"mt{rep}_{k}",
                                tag="mt")
                ot = opool.tile([NP, R, UOUT, 5], f32, name=f"ot{rep}_{k}",
                                tag="ot")
                mtf = mt.rearrange("p r u c -> p r (u c)")
                otf = ot.rearrange("p r u c -> p r (u c)")

                # ---- load 34-pixel band (pixels 32k-1 .. 32k+32, circular)
                a = (PIX * k - 1) * C
                if k == 0:
                    nc.sync.dma_start(mtf[:, :, C:FIN], xr[:, :, 0:FIN - C])
                    nc.sync.dma_start(mtf[:, :, 0:C],
                                      xr[:, :, (W - 1) * C:W * C])
                elif k == NCH - 1:
                    nc.sync.dma_start(mtf[:, :, 0:FIN - C],
                                      xr[:, :, a:a + FIN - C])
                    nc.sync.dma_start(mtf[:, :, FIN - C:FIN], xr[:, :, 0:C])
                else:
                    nc.sync.dma_start(mtf[:, :, :], xr[:, :, a:a + FIN])

                # ---- stage H-halo rows
                if variant == "dma":
                    ht = hpool.tile([NP, 2, UIN, 5], f32, name=f"ht{rep}_{k}",
                                    tag="ht")
                    htf = ht.rearrange("p s u c -> p s (u c)")
                    # slot 0: next row (4p+4) = partition p+1's row-slot 0
                    nc.sync.dma_start(htf[0:NP - 1, 0, :], mtf[1:NP, 0, :])
                    nc.sync.dma_start(htf[NP - 1:NP, 0, :], mtf[0:1, 0, :])
                    # slot 1: prev row (4p-1) = partition p-1's row-slot 3
                    nc.sync.dma_start(htf[1:NP, 1, :],
                                      mtf[0:NP - 1, R - 1, :])
                    nc.sync.dma_start(htf[0:1, 1, :],
                                      mtf[NP - 1:NP, R - 1, :])
                    nx = ht[:, 0, :, :]   # [NP, UIN, 5]
                    pv = ht[:, 1, :, :]
                else:
                    pn = ppool.tile([NP, 2048], f32, name=f"pn{rep}_{k}",
                                    tag="pn")
                    pp = ppool.tile([NP, 2048], f32, name=f"pp{rep}_{k}",
                                    tag="pp")
                    for j in range(4):
                        sz = min(512, FIN - 512 * j)
                        nc.tensor.matmul(pn[:, 512 * j:512 * j + sz], snt[:],
                                         mtf[:, 0, 512 * j:512 * j + sz])
                        nc.tensor.matmul(pp[:, 512 * j:512 * j + sz], spt[:],
                                         mtf[:, R - 1, 512 * j:512 * j + sz])
                    nx = pn[:, 0:FIN].rearrange("p (u c) -> p u c", c=5)
                    pv = pp[:, 0:FIN].rearrange("p (u c) -> p u c", c=5)

                # ---- assemble output residues (DVE strided copies)
                # r=0: w+1 -> in pixel-slot j+2 -> u offset +24
                nc.vector.tensor_copy(ot[:, :, :, 0], mt[:, :, 24:24 + UOUT, 0])
                # r=1: w-1 -> pixel-slot j -> u offset 0
                nc.vector.tensor_copy(ot[:, :, :, 1], mt[:, :, 0:UOUT, 1])
                # r=4: same pixel -> slot j+1 -> u offset +12
                nc.vector.tensor_copy(ot[:, :, :, 4], mt[:, :, 12:12 + UOUT, 4])
                # r=2: h+1 -> rows 0..2 from in rows 1..3
                nc.vector.tensor_copy(ot[:, 0:R - 1, :, 2],
                                      mt[:, 1:R, 12:12 + UOUT, 2])
                # r=2 row 3 from next-row halo
                nc.vector.tensor_copy(ot[:, R - 1, :, 2], nx[:, 12:12 + UOUT, 2])
                # r=3: h-1 -> rows 1..3 from in rows 0..2
                nc.vector.tensor_copy(ot[:, 1:R, :, 3],
                                      mt[:, 0:R - 1, 12:12 + UOUT, 3])
                # r=3 row 0 from prev-row halo
                nc.vector.tensor_copy(ot[:, 0, :, 3], pv[:, 12:12 + UOUT, 3])

                # ---- store
                nc.sync.dma_start(yr[:, :, k * FOUT:(k + 1) * FOUT],
                                  otf[:, :, :])

    nc.finalize()
    _NC_CACHE[key] = nc
    return nc


def make_in_maps(x, variant=VARIANT):
    B = x.shape[0]
    maps = [{"x": x[b]} for b in range(B)]
    if variant == "pe" or variant.startswith("v3"):
        sn, sp = shift_mats()
        for m in maps:
            m["sn"] = sn
            m["sp"] = sp
    return maps


def run(x: np.ndarray, variant=VARIANT):
    """Returns (out [B,H,W,C], BassKernelResults)."""
    x = np.ascontiguousarray(x, dtype=np.float32)
    B = x.shape[0]
    nc = _build_nc(variant)
    res = bass_utils.run_bass_kernel_spmd(nc, make_in_maps(x, variant),
                                          core_ids=list(range(B)))
    out = np.stack([r["y"] for r in res.results], axis=0)
    return out, res


def kernel(x: np.ndarray) -> np.ndarray:
    out, _ = run(x)
    return out

